# revision 8
# baseline (speedup 1.0000x reference)
"""Trainium2 Bass kernel for nn_Linear_24180665876920 (fp8 block-quant Linear).

Reference semantics (per core, data-parallel over rows of x):
  s[m, kb]  = max(amax(|x[m, kb*128:(kb+1)*128]|), 1e-4) / 448
  x_q       = fp8_e4m3fn(clip(x / s, +-448))
  x_deq     = x_q * s
  w_deq     = w_q * w_s   (per [128,128] block)
  out       = (x_deq @ w_deq.T).astype(bf16)

Device implementation notes:
  * TRN fp8_e4m3 max normal is +-240 (not OCP's +-448), so we quantize at
    half scale: fp8(x * (224/d)) * (d/224) == fp8_e4m3fn(x * (448/d)) * (d/448)
    up to fp32 rounding (power-of-2 rescale commutes with RNE rounding).
  * Matmul runs in bf16 (inputs rounded to bf16; PE multiplies exactly,
    accumulates ~fp32). Measured end-to-end error vs reference: ~0.6% of absmax.
  * Sharding: data-parallel over M (8192 rows -> 8 cores x 1024 rows).
    Weights are replicated; host pre-packs w_q losslessly to bf16 in a
    K-major tiled layout ([nt, kb, 128, 512]) so the kernel streams
    contiguous chunks; dequant (x w_s) happens on-device on the fly.
  * x is quantized in natural layout, bounced through DRAM, and read back
    with DMA-transpose into K-major SBUF-resident tiles for the matmul.
"""

import numpy as np
import ml_dtypes
from contextlib import ExitStack

import concourse.bass as bass
import concourse.mybir as mybir
import concourse.tile as tile
from concourse import bacc
from concourse.bass_utils import run_bass_kernel_spmd

P = 128
NCORES = 8
M, K, N = 8192, 7168, 4096
MC = M // NCORES            # 1024 rows of x per core
KB = K // P                 # 56 k-blocks
NT = N // 512               # 8 n-tiles of 512
MT = MC // P                # 8 m-tiles per core
G = 2                       # x_T groups (512 rows each)
HK = KB // 2                # 28 k-blocks per phase-A half tile
DT = mybir.dt
BF16 = DT.bfloat16
F32 = DT.float32


def _bcast(ap, n):
    """Append a stride-0 innermost dim of size n (broadcast along it)."""
    return bass.AP(tensor=ap.tensor, offset=ap.offset, ap=[*ap.ap, [0, n]])


def build_nc(reps=1):
    nc = bacc.Bacc(trn_type="TRN2")
    x_in = nc.dram_tensor("x", (MC, K), BF16, kind="ExternalInput")
    wt_in = nc.dram_tensor("wt", (NT, KB, P, 512), BF16, kind="ExternalInput")
    ws_in = nc.dram_tensor("ws", (P, KB * 32), F32, kind="ExternalInput")
    out = nc.dram_tensor("out", (MC, N), BF16, kind="ExternalOutput")
    xdq = nc.dram_tensor("xdq", (MC, K), BF16, kind="Internal")

    with tile.TileContext(nc) as tc, ExitStack() as ctx:
        singles = ctx.enter_context(tc.tile_pool(name="singles", bufs=1))
        xa_pool = ctx.enter_context(tc.tile_pool(name="xa", bufs=3))
        xq_pool = ctx.enter_context(tc.tile_pool(name="xqp", bufs=2))
        xd_pool = ctx.enter_context(tc.tile_pool(name="xdp", bufs=2))
        sc_pool = ctx.enter_context(tc.tile_pool(name="scales", bufs=3))
        w_pool = ctx.enter_context(tc.tile_pool(name="wp", bufs=4))
        o_pool = ctx.enter_context(tc.tile_pool(name="op", bufs=8))
        ps_pool = ctx.enter_context(tc.tile_pool(name="psp", bufs=1, space="PSUM"))
        xt_pool = ctx.enter_context(tc.tile_pool(name="xtp", bufs=1))

        ws_sb = singles.tile([P, KB * 32], F32)
        nc.sync.dma_start(out=ws_sb, in_=ws_in[:, :])

        for _rep in range(reps):
            _build_body(nc, tc, x_in, wt_in, ws_sb, out, xdq,
                        xa_pool, xq_pool, xd_pool, sc_pool, w_pool, o_pool,
                        ps_pool, xt_pool)
    return nc


def _build_body(nc, tc, x_in, wt_in, ws_sb, out, xdq,
                xa_pool, xq_pool, xd_pool, sc_pool, w_pool, o_pool,
                ps_pool, xt_pool):
    if True:
        xt = [
            xt_pool.tile([P, KB, 512], BF16, name=f"xt{g}", tag=f"xt{g}")
            for g in range(G)
        ]

        # ---- Phase A: activation quant/dequant (natural layout), store to DRAM;
        # ---- Phase B: transposed read-back into resident K-major tiles.
        for g in range(G):
            for mt in range(g * MT // G, (g + 1) * MT // G):
                rows = slice(mt * P, (mt + 1) * P)
                for h in range(2):
                    cols = slice(h * HK * P, (h + 1) * HK * P)
                    xa = xa_pool.tile([P, HK, P], BF16, name="xa", tag="xa")
                    nc.sync.dma_start(
                        out=xa,
                        in_=x_in[rows, cols].rearrange("p (b c) -> p b c", c=P),
                    )
                    am = sc_pool.tile([P, HK], F32, name="am", tag="am")
                    nc.vector.reduce_max(
                        out=am, in_=xa, axis=mybir.AxisListType.X,
                        apply_absolute_value=True,
                    )
                    nc.vector.tensor_scalar_max(out=am, in0=am, scalar1=1e-4)
                    tt = sc_pool.tile([P, HK], F32, name="tt", tag="tt")
                    nc.vector.tensor_scalar_mul(out=tt, in0=am, scalar1=1.0 / 224.0)
                    rr = sc_pool.tile([P, HK], F32, name="rr", tag="rr")
                    nc.vector.reciprocal(out=rr, in_=tt)
                    xq = xq_pool.tile([P, HK, P], DT.float8e4, name="xq", tag="xq")
                    nc.vector.tensor_mul(out=xq, in0=xa, in1=_bcast(rr, P))
                    xd = xd_pool.tile([P, HK, P], BF16, name="xd", tag="xd")
                    # dequant on GPSIMD: frees DVE for the w-dequant stream
                    nc.gpsimd.tensor_mul(out=xd, in0=xq, in1=_bcast(tt, P))
                    nc.sync.dma_start(
                        out=xdq[rows, cols].rearrange("p (b c) -> p b c", c=P),
                        in_=xd,
                    )
            # Transposed loads for this group (ACT's HWDGE ring, separate
            # FIFO from the sync/SP ring carrying the bulk loads).
            for kb in range(KB):
                nc.scalar.dma_start(
                    out=xt[g][:, kb, :],
                    in_=xdq[g * 512:(g + 1) * 512, kb * P:(kb + 1) * P],
                    transpose=True,
                )

        # ---- Phase C: matmul. out[m, n] = sum_kb xT[kb,:,m].T @ w_deq[kb, n]
        for nt in range(NT):
            ps = [
                ps_pool.tile([P, 512], F32, name=f"ps{i}", tag=f"ps{i}")
                for i in range(8)
            ]
            for kb in range(KB):
                wq = w_pool.tile([P, 4, P], BF16, name="wq", tag="wq")
                nc.sync.dma_start(
                    out=wq,
                    in_=wt_in[nt, kb].rearrange("p (b c) -> p b c", c=P),
                )
                wd = w_pool.tile([P, 4, P], BF16, name="wd", tag="wd")
                base = ws_sb[:, kb * 32 + 4 * nt: kb * 32 + 4 * nt + 4]
                nc.vector.tensor_mul(out=wd, in0=wq, in1=_bcast(base, P))
                wd2 = wd.rearrange("p b c -> p (b c)")
                first, last = (kb == 0), (kb == KB - 1)
                for g in range(G):
                    for mi in range(4):
                        nc.tensor.matmul(
                            ps[g * 4 + mi],
                            lhsT=xt[g][:, kb, mi * P:(mi + 1) * P],
                            rhs=wd2,
                            start=first,
                            stop=last,
                        )
            for i in range(8):
                g, mi = divmod(i, 4)
                row0 = g * 512 + mi * P
                ot = o_pool.tile([P, 512], BF16, name="ot", tag="ot")
                nc.scalar.copy(out=ot, in_=ps[i])
                nc.sync.dma_start(
                    out=out[row0:row0 + P, nt * 512:(nt + 1) * 512], in_=ot
                )


def _prep_host_inputs(x, w_q, w_s):
    x = np.asarray(x)
    if x.dtype != ml_dtypes.bfloat16:
        x = x.astype(ml_dtypes.bfloat16)
    w_q = np.asarray(w_q, dtype=np.float32)
    w_s = np.asarray(w_s, dtype=np.float32)
    # w_q values are fp8-representable -> bf16 conversion is lossless.
    wt = np.ascontiguousarray(w_q.astype(ml_dtypes.bfloat16).T)      # [K, N]
    wtl = np.ascontiguousarray(
        wt.reshape(KB, P, NT, 512).transpose(2, 0, 1, 3)             # [NT,KB,P,512]
    )
    # ws[p, kb*32 + nb] = w_s[nb, kb], replicated across partitions
    ws_host = np.ascontiguousarray(
        np.broadcast_to(w_s.T.reshape(1, KB * 32), (P, KB * 32))
    ).astype(np.float32)
    return x, wtl, ws_host


def kernel(x, w_q, w_s, _trace=False, _tmpdir=None):
    x, wtl, ws_host = _prep_host_inputs(x, w_q, w_s)
    nc = build_nc()
    if not nc.is_finalized():
        nc.finalize()
    in_maps = [
        {"x": x[c * MC:(c + 1) * MC], "wt": wtl, "ws": ws_host}
        for c in range(NCORES)
    ]
    res = run_bass_kernel_spmd(
        nc, in_maps, core_ids=list(range(NCORES)),
        trace=_trace, tmpdir=_tmpdir,
    )
    out = np.concatenate([r["out"] for r in res.results], axis=0)
    if _trace:
        return out, res
    return out


if __name__ == "__main__":
    rng = np.random.default_rng(0)
    x = rng.standard_normal((M, K), dtype=np.float32).astype(ml_dtypes.bfloat16)
    w = rng.standard_normal((N, K), dtype=np.float32).astype(np.float32) * 0.02
    w_s = np.abs(rng.standard_normal((N // P, K // P), dtype=np.float32)) * 0.01 + 1e-3
    out = kernel(x, w, w_s)
    print("ran ok:", out.shape, out.dtype)


# revision 22
# speedup vs baseline: 8.0048x; 8.0048x over previous
"""Trainium2 Bass kernel for nn_Linear_24180665876920 (fp8 block-quant Linear).

Reference semantics (per core, data-parallel over rows of x):
  s[m, kb]  = max(amax(|x[m, kb*128:(kb+1)*128]|), 1e-4) / 448
  x_q       = fp8_e4m3fn(clip(x / s, +-448))
  x_deq     = x_q * s
  w_deq     = w_q * w_s   (per [128,128] block)
  out       = (x_deq @ w_deq.T).astype(bf16)

Device implementation notes:
  * TRN fp8_e4m3 max normal is +-240 (not OCP's +-448), so we quantize at
    half scale: fp8(x * (224/d)) * (d/224) == fp8_e4m3fn(x * (448/d)) * (d/448)
    up to fp32 rounding (power-of-2 rescale commutes with RNE rounding).
  * Matmul runs in bf16 (inputs rounded to bf16; PE multiplies exactly,
    accumulates ~fp32). Measured end-to-end error vs reference: ~0.6% of absmax.
  * Sharding: data-parallel over M (8192 rows -> 8 cores x 1024 rows).
    Weights are replicated; host pre-packs w_q losslessly to bf16 in a
    K-major tiled layout ([nt, kb, 128, 512]) so the kernel streams
    contiguous chunks; dequant (x w_s) happens on-device on the fly.
  * x is quantized in natural layout, bounced through DRAM, and read back
    with DMA-transpose into K-major SBUF-resident tiles for the matmul.
"""

import numpy as np
import ml_dtypes
from contextlib import ExitStack

import concourse.bass as bass
import concourse.mybir as mybir
import concourse.tile as tile
from concourse import bacc
from concourse.bass_utils import run_bass_kernel_spmd

P = 128
NCORES = 8
M, K, N = 8192, 7168, 4096
MC = M // NCORES            # 1024 rows of x per core
KB = K // P                 # 56 k-blocks
NT = N // 512               # 8 n-tiles of 512
MT = MC // P                # 8 m-tiles per core
G = 2                       # x_T groups (512 rows each)
HK = KB // 2                # 28 k-blocks per phase-A half tile
DT = mybir.dt
BF16 = DT.bfloat16
F32 = DT.float32


def _bcast(ap, n):
    """Append a stride-0 innermost dim of size n (broadcast along it)."""
    return bass.AP(tensor=ap.tensor, offset=ap.offset, ap=[*ap.ap, [0, n]])


def build_nc(reps=1):
    nc = bacc.Bacc(trn_type="TRN2")
    x_in = nc.dram_tensor("x", (MC, K), BF16, kind="ExternalInput")
    wt_in = nc.dram_tensor("wt", (NT, KB, P, 512), BF16, kind="ExternalInput")
    ws_in = nc.dram_tensor("ws", (P, KB * 32), F32, kind="ExternalInput")
    out = nc.dram_tensor("out", (MC, N), BF16, kind="ExternalOutput")

    with tile.TileContext(nc) as tc, ExitStack() as ctx:
        singles = ctx.enter_context(tc.tile_pool(name="singles", bufs=1))
        xa_pool = ctx.enter_context(tc.tile_pool(name="xa", bufs=2))
        xq_pool = ctx.enter_context(tc.tile_pool(name="xqp", bufs=2))
        xd_pool = ctx.enter_context(tc.tile_pool(name="xdp", bufs=2))
        sc_pool = ctx.enter_context(tc.tile_pool(name="scales", bufs=3))
        w_pool = ctx.enter_context(tc.tile_pool(name="wp", bufs=3))
        o_pool = ctx.enter_context(tc.tile_pool(name="op", bufs=2))
        ps_pool = ctx.enter_context(tc.tile_pool(name="psp", bufs=1, space="PSUM"))
        xt_pool = ctx.enter_context(tc.tile_pool(name="xtp", bufs=1))

        ws_sb = singles.tile([P, KB * 32], F32)
        nc.sync.dma_start(out=ws_sb, in_=ws_in[:, :])

        for _rep in range(reps):
            _build_body(nc, tc, x_in, wt_in, ws_sb, out,
                        xa_pool, xq_pool, xd_pool, sc_pool, w_pool, o_pool,
                        ps_pool, xt_pool)
    return nc


def _build_body(nc, tc, x_in, wt_in, ws_sb, out,
                xa_pool, xq_pool, xd_pool, sc_pool, w_pool, o_pool,
                ps_pool, xt_pool):
    if True:
        xt = [
            xt_pool.tile([P, KB, 512], BF16, name=f"xt{g}", tag=f"xt{g}")
            for g in range(G)
        ]

        # ---- Phase A: activation quant/dequant (natural layout), then
        # per-block SBUF->SBUF DMA transposes into resident K-major tiles.
        for g in range(G):
            for mt in range(g * MT // G, (g + 1) * MT // G):
                mi = mt - g * (MT // G)
                rows = slice(mt * P, (mt + 1) * P)
                for h in range(2):
                    cols = slice(h * HK * P, (h + 1) * HK * P)
                    xa = xa_pool.tile([P, HK, P], BF16, name="xa", tag="xa")
                    nc.sync.dma_start(
                        out=xa,
                        in_=x_in[rows, cols].rearrange("p (b c) -> p b c", c=P),
                    )
                    am = sc_pool.tile([P, HK], F32, name="am", tag="am")
                    nc.vector.reduce_max(
                        out=am, in_=xa, axis=mybir.AxisListType.X,
                        apply_absolute_value=True,
                    )
                    nc.vector.tensor_scalar_max(out=am, in0=am, scalar1=1e-4)
                    tt = sc_pool.tile([P, HK], F32, name="tt", tag="tt")
                    nc.vector.tensor_scalar_mul(out=tt, in0=am, scalar1=1.0 / 224.0)
                    rr = sc_pool.tile([P, HK], F32, name="rr", tag="rr")
                    nc.vector.reciprocal(out=rr, in_=tt)
                    xq = xq_pool.tile([P, HK, P], DT.float8e4, name="xq", tag="xq")
                    nc.vector.tensor_mul(out=xq, in0=xa, in1=_bcast(rr, P))
                    xd = xd_pool.tile([P, HK, P], BF16, name="xd", tag="xd")
                    # dequant on GPSIMD: frees DVE for the w-dequant stream
                    nc.gpsimd.tensor_mul(out=xd, in0=xq, in1=_bcast(tt, P))
                    # One SBUF->SBUF xbar transpose for the whole half-tile:
                    # [128m, 28*128k] -> [128k, 28kb, 128m] directly into the
                    # K-major resident tile (ACT's HWDGE ring).
                    nc.scalar.dma_start(
                        out=xt[g][:, h * HK:(h + 1) * HK, mi * P:(mi + 1) * P],
                        in_=xd.rearrange("p b c -> p (b c)"),
                        transpose=True,
                    )

        # ---- Phase C: matmul. out[m, n] = sum_kb xT[kb,:,m].T @ w_deq[kb, n]
        KBB = 4  # k-blocks per w chunk-group (one DMA + one dequant op each)

        def kb_loop(nt, ps, groups):
            for kb0 in range(0, KB, KBB):
                wq = w_pool.tile([P, KBB, 512], BF16, name="wq", tag="wq")
                nc.sync.dma_start(
                    out=wq, in_=wt_in[nt, kb0:kb0 + KBB].rearrange("b p c -> p b c"),
                )
                wd = w_pool.tile([P, KBB, 512], BF16, name="wd", tag="wd")
                sc = ws_sb[:, :]
                scb = bass.AP(
                    tensor=sc.tensor,
                    offset=sc.offset + kb0 * 32 + 4 * nt,
                    ap=[sc.ap[0], [32, KBB], [1, 4], [0, P]],
                )
                nc.vector.tensor_mul(
                    out=wd.rearrange("p b (n c) -> p b n c", c=P), in0=wq.rearrange("p b (n c) -> p b n c", c=P), in1=scb,
                )
                for j in range(KBB):
                    kb = kb0 + j
                    first, last = (kb == 0), (kb == KB - 1)
                    for g in groups:
                        for mi in range(4):
                            nc.tensor.matmul(
                                ps[g * 4 + mi],
                                lhsT=xt[g][:, kb, mi * P:(mi + 1) * P],
                                rhs=wd[:, j, :],
                                start=first,
                                stop=last,
                            )

        for nt in range(NT):
            ps = [
                ps_pool.tile([P, 512], F32, name=f"ps{i}", tag=f"ps{i}")
                for i in range(8)
            ]
            if nt == 0:
                # Split by m-group so the PE starts as soon as the first
                # 512 rows of x are quantized (w chunks loaded twice here).
                kb_loop(nt, ps, [0])
                kb_loop(nt, ps, [1])
            else:
                kb_loop(nt, ps, list(range(G)))
            for g in range(G):
                ot = o_pool.tile([P, 4, 512], BF16, name="ot", tag="ot")
                for mi in range(4):
                    nc.scalar.copy(out=ot[:, mi, :], in_=ps[g * 4 + mi])
                nc.sync.dma_start(
                    out=out[g * 512:(g + 1) * 512, nt * 512:(nt + 1) * 512]
                    .rearrange("(b p) c -> p b c", p=P),
                    in_=ot,
                )


def _prep_host_inputs(x, w_q, w_s):
    x = np.asarray(x)
    if x.dtype != ml_dtypes.bfloat16:
        x = x.astype(ml_dtypes.bfloat16)
    w_q = np.asarray(w_q, dtype=np.float32)
    w_s = np.asarray(w_s, dtype=np.float32)
    # w_q values are fp8-representable -> bf16 conversion is lossless.
    wt = np.ascontiguousarray(w_q.astype(ml_dtypes.bfloat16).T)      # [K, N]
    wtl = np.ascontiguousarray(
        wt.reshape(KB, P, NT, 512).transpose(2, 0, 1, 3)             # [NT,KB,P,512]
    )
    # ws[p, kb*32 + nb] = w_s[nb, kb], replicated across partitions
    ws_host = np.ascontiguousarray(
        np.broadcast_to(w_s.T.reshape(1, KB * 32), (P, KB * 32))
    ).astype(np.float32)
    return x, wtl, ws_host


def kernel(x, w_q, w_s, _trace=False, _tmpdir=None):
    x, wtl, ws_host = _prep_host_inputs(x, w_q, w_s)
    nc = build_nc()
    if not nc.is_finalized():
        nc.finalize()
    in_maps = [
        {"x": x[c * MC:(c + 1) * MC], "wt": wtl, "ws": ws_host}
        for c in range(NCORES)
    ]
    res = run_bass_kernel_spmd(
        nc, in_maps, core_ids=list(range(NCORES)),
        trace=_trace, tmpdir=_tmpdir,
    )
    out = np.concatenate([r["out"] for r in res.results], axis=0)
    if _trace:
        return out, res
    return out


if __name__ == "__main__":
    rng = np.random.default_rng(0)
    x = rng.standard_normal((M, K), dtype=np.float32).astype(ml_dtypes.bfloat16)
    w = rng.standard_normal((N, K), dtype=np.float32).astype(np.float32) * 0.02
    w_s = np.abs(rng.standard_normal((N // P, K // P), dtype=np.float32)) * 0.01 + 1e-3
    out = kernel(x, w, w_s)
    print("ran ok:", out.shape, out.dtype)


# revision 23
# speedup vs baseline: 503.7343x; 62.9288x over previous
"""Trainium2 Bass kernel for nn_Linear_24180665876920 (fp8 block-quant Linear).

Reference semantics (per core, data-parallel over rows of x):
  s[m, kb]  = max(amax(|x[m, kb*128:(kb+1)*128]|), 1e-4) / 448
  x_q       = fp8_e4m3fn(clip(x / s, +-448))
  x_deq     = x_q * s
  w_deq     = w_q * w_s   (per [128,128] block)
  out       = (x_deq @ w_deq.T).astype(bf16)

Device implementation notes:
  * TRN fp8_e4m3 max normal is +-240 (not OCP's +-448), so we quantize at
    half scale: fp8(x * (224/d)) * (d/224) == fp8_e4m3fn(x * (448/d)) * (d/448)
    up to fp32 rounding (power-of-2 rescale commutes with RNE rounding).
  * Matmul runs in bf16 (inputs rounded to bf16; PE multiplies exactly,
    accumulates ~fp32). Measured end-to-end error vs reference: ~0.6% of absmax.
  * Sharding: data-parallel over M (8192 rows -> 8 cores x 1024 rows).
    Weights are replicated; host pre-packs w_q losslessly to bf16 in a
    K-major tiled layout ([nt, kb, 128, 512]) so the kernel streams
    contiguous chunks; dequant (x w_s) happens on-device on the fly.
  * x is quantized in natural layout, bounced through DRAM, and read back
    with DMA-transpose into K-major SBUF-resident tiles for the matmul.
"""

import numpy as np
import ml_dtypes
from contextlib import ExitStack

import concourse.bass as bass
import concourse.mybir as mybir
import concourse.tile as tile
from concourse import bacc
from concourse.bass_utils import run_bass_kernel_spmd

P = 128
NCORES = 8
M, K, N = 8192, 7168, 4096
MC = M // NCORES            # 1024 rows of x per core
KB = K // P                 # 56 k-blocks
NT = N // 512               # 8 n-tiles of 512
MT = MC // P                # 8 m-tiles per core
G = 2                       # x_T groups (512 rows each)
HK = KB // 2                # 28 k-blocks per phase-A half tile
DT = mybir.dt
BF16 = DT.bfloat16
F32 = DT.float32


def _bcast(ap, n):
    """Append a stride-0 innermost dim of size n (broadcast along it)."""
    return bass.AP(tensor=ap.tensor, offset=ap.offset, ap=[*ap.ap, [0, n]])


def build_nc(reps=1):
    nc = bacc.Bacc(trn_type="TRN2")
    x_in = nc.dram_tensor("x", (MC, K), BF16, kind="ExternalInput")
    wt_in = nc.dram_tensor("wt", (NT, KB, P, 512), BF16, kind="ExternalInput")
    ws_in = nc.dram_tensor("ws", (P, KB * 32), F32, kind="ExternalInput")
    out = nc.dram_tensor("out", (MC, N), BF16, kind="ExternalOutput")

    with tile.TileContext(nc) as tc, ExitStack() as ctx:
        singles = ctx.enter_context(tc.tile_pool(name="singles", bufs=1))
        xa_pool = ctx.enter_context(tc.tile_pool(name="xa", bufs=2))
        xq_pool = ctx.enter_context(tc.tile_pool(name="xqp", bufs=2))
        xd_pool = ctx.enter_context(tc.tile_pool(name="xdp", bufs=2))
        sc_pool = ctx.enter_context(tc.tile_pool(name="scales", bufs=3))
        w_pool = ctx.enter_context(tc.tile_pool(name="wp", bufs=3))
        o_pool = ctx.enter_context(tc.tile_pool(name="op", bufs=2))
        ps_pool = ctx.enter_context(tc.tile_pool(name="psp", bufs=1, space="PSUM"))
        xt_pool = ctx.enter_context(tc.tile_pool(name="xtp", bufs=1))

        ws_sb = singles.tile([P, KB * 32], F32)
        nc.sync.dma_start(out=ws_sb, in_=ws_in[:, :])

        for _rep in range(reps):
            _build_body(nc, tc, x_in, wt_in, ws_sb, out,
                        xa_pool, xq_pool, xd_pool, sc_pool, w_pool, o_pool,
                        ps_pool, xt_pool)
    return nc


def _build_body(nc, tc, x_in, wt_in, ws_sb, out,
                xa_pool, xq_pool, xd_pool, sc_pool, w_pool, o_pool,
                ps_pool, xt_pool):
    if True:
        xt = [
            xt_pool.tile([P, KB, 512], BF16, name=f"xt{g}", tag=f"xt{g}")
            for g in range(G)
        ]

        # ---- Phase A: activation quant/dequant (natural layout), then
        # per-block SBUF->SBUF DMA transposes into resident K-major tiles.
        for g in range(G):
            # h outer: the first 4 half-tiles cover kb 0..27 for every m-tile
            # of the group, so the PE's kb sweep can start ~2x earlier.
            for h in range(2):
                for mt in range(g * MT // G, (g + 1) * MT // G):
                    mi = mt - g * (MT // G)
                    rows = slice(mt * P, (mt + 1) * P)
                    cols = slice(h * HK * P, (h + 1) * HK * P)
                    xa = xa_pool.tile([P, HK, P], BF16, name="xa", tag="xa")
                    nc.sync.dma_start(
                        out=xa,
                        in_=x_in[rows, cols].rearrange("p (b c) -> p b c", c=P),
                    )
                    am = sc_pool.tile([P, HK], F32, name="am", tag="am")
                    nc.vector.reduce_max(
                        out=am, in_=xa, axis=mybir.AxisListType.X,
                        apply_absolute_value=True,
                    )
                    tt = sc_pool.tile([P, HK], F32, name="tt", tag="tt")
                    nc.vector.tensor_scalar(
                        out=tt, in0=am, scalar1=1e-4, scalar2=1.0 / 224.0,
                        op0=mybir.AluOpType.max, op1=mybir.AluOpType.mult,
                    )
                    rr = sc_pool.tile([P, HK], F32, name="rr", tag="rr")
                    nc.vector.reciprocal(out=rr, in_=tt)
                    xq = xq_pool.tile([P, HK, P], DT.float8e4, name="xq", tag="xq")
                    nc.vector.tensor_mul(out=xq, in0=xa, in1=_bcast(rr, P))
                    xd = xd_pool.tile([P, HK, P], BF16, name="xd", tag="xd")
                    # dequant on GPSIMD: frees DVE for the w-dequant stream
                    nc.gpsimd.tensor_mul(out=xd, in0=xq, in1=_bcast(tt, P))
                    # One SBUF->SBUF xbar transpose for the whole half-tile:
                    # [128m, 28*128k] -> [128k, 28kb, 128m] directly into the
                    # K-major resident tile (ACT's HWDGE ring).
                    nc.scalar.dma_start(
                        out=xt[g][:, h * HK:(h + 1) * HK, mi * P:(mi + 1) * P],
                        in_=xd.rearrange("p b c -> p (b c)"),
                        transpose=True,
                    )

        # ---- Phase C: matmul. out[m, n] = sum_kb xT[kb,:,m].T @ w_deq[kb, n]
        KBB = 4  # k-blocks per w chunk-group (one DMA + one dequant op each)

        def kb_loop(nt, ps, groups):
            for kb0 in range(0, KB, KBB):
                wq = w_pool.tile([P, KBB, 512], BF16, name="wq", tag="wq")
                nc.sync.dma_start(
                    out=wq, in_=wt_in[nt, kb0:kb0 + KBB].rearrange("b p c -> p b c"),
                )
                wd = w_pool.tile([P, KBB, 512], BF16, name="wd", tag="wd")
                sc = ws_sb[:, :]
                scb = bass.AP(
                    tensor=sc.tensor,
                    offset=sc.offset + kb0 * 32 + 4 * nt,
                    ap=[sc.ap[0], [32, KBB], [1, 4], [0, P]],
                )
                nc.vector.tensor_mul(
                    out=wd.rearrange("p b (n c) -> p b n c", c=P), in0=wq.rearrange("p b (n c) -> p b n c", c=P), in1=scb,
                )
                for j in range(KBB):
                    kb = kb0 + j
                    first, last = (kb == 0), (kb == KB - 1)
                    for g in groups:
                        for mi in range(4):
                            nc.tensor.matmul(
                                ps[g * 4 + mi],
                                lhsT=xt[g][:, kb, mi * P:(mi + 1) * P],
                                rhs=wd[:, j, :],
                                start=first,
                                stop=last,
                            )

        for nt in range(NT):
            ps = [
                ps_pool.tile([P, 512], F32, name=f"ps{i}", tag=f"ps{i}")
                for i in range(8)
            ]
            if nt == 0:
                # Split by m-group so the PE starts as soon as the first
                # 512 rows of x are quantized (w chunks loaded twice here).
                kb_loop(nt, ps, [0])
                kb_loop(nt, ps, [1])
            else:
                kb_loop(nt, ps, list(range(G)))
            for g in range(G):
                ot = o_pool.tile([P, 4, 512], BF16, name="ot", tag="ot")
                for mi in range(4):
                    nc.scalar.copy(out=ot[:, mi, :], in_=ps[g * 4 + mi])
                nc.sync.dma_start(
                    out=out[g * 512:(g + 1) * 512, nt * 512:(nt + 1) * 512]
                    .rearrange("(b p) c -> p b c", p=P),
                    in_=ot,
                )


def _prep_host_inputs(x, w_q, w_s):
    x = np.asarray(x)
    if x.dtype != ml_dtypes.bfloat16:
        x = x.astype(ml_dtypes.bfloat16)
    w_q = np.asarray(w_q, dtype=np.float32)
    w_s = np.asarray(w_s, dtype=np.float32)
    # w_q values are fp8-representable -> bf16 conversion is lossless.
    wt = np.ascontiguousarray(w_q.astype(ml_dtypes.bfloat16).T)      # [K, N]
    wtl = np.ascontiguousarray(
        wt.reshape(KB, P, NT, 512).transpose(2, 0, 1, 3)             # [NT,KB,P,512]
    )
    # ws[p, kb*32 + nb] = w_s[nb, kb], replicated across partitions
    ws_host = np.ascontiguousarray(
        np.broadcast_to(w_s.T.reshape(1, KB * 32), (P, KB * 32))
    ).astype(np.float32)
    return x, wtl, ws_host


def kernel(x, w_q, w_s, _trace=False, _tmpdir=None):
    x, wtl, ws_host = _prep_host_inputs(x, w_q, w_s)
    nc = build_nc()
    if not nc.is_finalized():
        nc.finalize()
    in_maps = [
        {"x": x[c * MC:(c + 1) * MC], "wt": wtl, "ws": ws_host}
        for c in range(NCORES)
    ]
    res = run_bass_kernel_spmd(
        nc, in_maps, core_ids=list(range(NCORES)),
        trace=_trace, tmpdir=_tmpdir,
    )
    out = np.concatenate([r["out"] for r in res.results], axis=0)
    if _trace:
        return out, res
    return out


if __name__ == "__main__":
    rng = np.random.default_rng(0)
    x = rng.standard_normal((M, K), dtype=np.float32).astype(ml_dtypes.bfloat16)
    w = rng.standard_normal((N, K), dtype=np.float32).astype(np.float32) * 0.02
    w_s = np.abs(rng.standard_normal((N // P, K // P), dtype=np.float32)) * 0.01 + 1e-3
    out = kernel(x, w, w_s)
    print("ran ok:", out.shape, out.dtype)
